# revision 15
# baseline (speedup 1.0000x reference)
"""Bahdanau additive attention kernel for Trainium2, data-parallel over 8 NeuronCores.

reference:
    W_hidden = hidden @ W_w.T + W_b                      # [A]
    U_encode = encoder_outputs @ U_w.T + U_b             # [S, A]
    poly_tanh = tanh(U_encode + W_hidden)                # [S, A]
    scores = poly_tanh @ V_w.T + V_b                     # [S, 1]
    attn = softmax(scores, axis=0)
    context = attn.T @ encoder_outputs                   # -> [1, 1, H]

Sharding: encoder_outputs split along S across the 8 cores; the small
U/W/V weights are replicated.  Each core emits an unnormalized partial
context sum(p_s * enc[s, :]) and partial denominator sum(p_s) with
p_s = exp(score_s - c) for a host-chosen constant shift c (softmax is
shift-invariant, so V_b is dropped and c only guards against overflow;
tanh bounds |score| <= sum|V_w|, so c is computable on the host without
touching the scores).  The host reduces the 8 partials — the one
"all-reduce" this graph needs.

Per-core structure (S_loc = 1024, split into 4 s-chunks of 256):
  U_wT stays SBUF-resident ([128h, kt*a*128] = 16.8 MB).  Its 32 per-kt
  DMAs interleave with chunk 0's encT so the PE ramps while weights
  stream (chunk 0 walks kt-outer to match arrival order; later chunks
  walk a-outer and are fully dense).  encT is streamed per chunk in a
  host-packed [128p, kt*256s] layout (32 KB contiguous DMA lines).
  Per chunk: 256 f32r matmuls (K=h on partitions, N=256 => 1 cycle/row)
  accumulate UE[a] in per-a single-bank PSUM tiles; bias+tanh fused on
  ScalarE (a on partitions => per-partition bias); scores via M=1 f32r
  matmuls software-pipelined one a-group behind the main stream; exp on
  ScalarE; p broadcast across partitions on GpSimd; context = per-kt
  fused multiply-reduce (affine_mul_reduce) on VectorE against the
  chunk's encT, hidden under the next chunk's matmuls.  Only the last
  chunk's softmax+context is tail-exposed.
"""

import os
import sys
from contextlib import ExitStack

if "/opt/trn_rl_repo" not in sys.path:
    sys.path.insert(0, "/opt/trn_rl_repo")

import numpy as np

S, H, A, NCORES = 8192, 4096, 1024, 8
S_LOC = S // NCORES  # 1024
P = 128
KT = H // P  # 32 k-tiles (contraction over h)
AT = A // P  # 8 a-tiles
NQ = 4  # s-chunks per core
QW = S_LOC // NQ  # 256 (>= 256 keeps f32r matmuls at 1 cycle/row)

MODE = os.environ.get("NN_ATTN_MODE", "f32r")  # f32r | f32
TRACE = bool(int(os.environ.get("NN_ATTN_TRACE", "0")))

LAST_EXEC_NS = None
LAST_RESULTS = None

_COMPILED = {}


def _build(mode):
    import concourse.bacc as bacc
    import concourse.mybir as mybir
    import concourse.tile as tile

    dt = mybir.dt
    mm_dt = dt.float32r if mode == "f32r" else dt.float32

    nc = bacc.Bacc("TRN2", target_bir_lowering=False, debug=False)

    # enc_q[q][p][kt*QW + s] = enc[q*QW + s, kt*128 + p] — per-chunk slabs
    # with 32KB-contiguous per-partition DMA lines.
    enc_q = nc.dram_tensor("enc_q", [NQ, P, KT * QW], mm_dt, kind="ExternalInput").ap()
    u_r = nc.dram_tensor("u_r", [P, KT, AT, P], mm_dt, kind="ExternalInput").ap()
    bias_a = nc.dram_tensor("bias_a", [AT, P], dt.float32, kind="ExternalInput").ap()
    v_w = nc.dram_tensor("v_w", [AT, P], mm_dt, kind="ExternalInput").ap()
    neg_c = nc.dram_tensor("neg_c", [1, 1], dt.float32, kind="ExternalInput").ap()
    # ctx_out[p][q*KT + kt] = partial ctx of chunk q for h = kt*128 + p
    ctx_o = nc.dram_tensor("ctx_out", [P, NQ * KT], dt.float32, kind="ExternalOutput").ap()
    l_o = nc.dram_tensor("l_out", [1, 1], dt.float32, kind="ExternalOutput").ap()

    with tile.TileContext(nc) as tc, ExitStack() as ctx:
        const = ctx.enter_context(tc.tile_pool(name="const", bufs=1))
        u_pool = ctx.enter_context(tc.tile_pool(name="u", bufs=1))
        et_pool = ctx.enter_context(tc.tile_pool(name="et", bufs=2))
        t_pool = ctx.enter_context(tc.tile_pool(name="t", bufs=3))
        pbc_pool = ctx.enter_context(tc.tile_pool(name="pbc", bufs=2))
        out_pool = ctx.enter_context(tc.tile_pool(name="out", bufs=1))
        ps_ue = ctx.enter_context(tc.tile_pool(name="ps_ue", bufs=5, space="PSUM"))
        ps_sc = ctx.enter_context(tc.tile_pool(name="ps_sc", bufs=2, space="PSUM"))
        ps_warm = ctx.enter_context(tc.tile_pool(name="ps_warm", bufs=1, space="PSUM"))

        bias_sb = const.tile([P, AT], dt.float32)
        v_sb = const.tile([P, AT], mm_dt)
        negc_sb = const.tile([1, 1], dt.float32)

        p_row = const.tile([1, S_LOC], dt.float32)
        ctx_sb = out_pool.tile([P, NQ * KT], dt.float32)
        scratch = [
            out_pool.tile([P, QW], dt.float32, name=f"scratch{i}") for i in range(2)
        ]
        scratch2 = out_pool.tile([P, QW], dt.float32)

        # resident U_wT [128h, (kt, a, 128a)]: 32 per-kt DMAs interleaved
        # with chunk 0's encT sub-loads on the SP queues, so data needed by
        # the kt-outer chunk-0 sweep arrives in consumption order.
        # PE pre-warm: ~64 dummy matmuls on a zeroed scratch tile keep the
        # tensor engine busy during the initial DMA wait so the HAM clock
        # gate reaches 2.4 GHz before the real matmuls start.
        warm_sb = const.tile([P, P + 64], dt.float32)
        nc.vector.memset(warm_sb[:], 0.0)
        pwarm = ps_warm.tile([P, 64], dt.float32)
        for _ in range(48):
            nc.tensor.matmul(
                pwarm[:], warm_sb[:, 0:P], warm_sb[:, P : P + 64],
                start=True, stop=True,
            )

        u_sb = u_pool.tile([P, KT * AT * P], mm_dt)
        et0 = et_pool.tile([P, KT * QW], mm_dt, tag="et", name="et_q0")
        for kt in range(KT):
            nc.sync.dma_start(u_sb[:, kt * AT * P : (kt + 1) * AT * P], u_r[:, kt])
            nc.sync.dma_start(
                et0[:, kt * QW : (kt + 1) * QW],
                enc_q[0][:, kt * QW : (kt + 1) * QW],
            )
            if kt == 1:
                # small inputs ride along after the first two k-slabs
                nc.sync.dma_start(bias_sb[:], bias_a.rearrange("at p -> p at"))
                nc.sync.dma_start(v_sb[:], v_w.rearrange("at p -> p at"))
                nc.sync.dma_start(negc_sb[:], neg_c[:])

        def mm_group(pue, et, a, kt):
            nc.tensor.matmul(
                pue[:, 0:QW],
                u_sb[:, (kt * AT + a) * P : (kt * AT + a + 1) * P],
                et[:, kt * QW : (kt + 1) * QW],
                start=(kt == 0),
                stop=(kt == KT - 1),
            )

        deferred = []  # score-matmul thunks, flushed one a-group behind

        def flush_deferred():
            while deferred:
                deferred.pop(0)()

        for q in range(NQ):
            if q == 0:
                et = et0
            else:
                et = et_pool.tile([P, KT * QW], mm_dt, tag="et", name=f"et_q{q}")
                # later chunks prefetch on the Activation HWDGE queues, off
                # the critical SP stream
                nc.scalar.dma_start(et[:], enc_q[q])
            psc = ps_sc.tile([1, QW], dt.float32, tag="psc", name=f"psc_q{q}")
            pues = {}
            for a in range(AT):
                pues[a] = ps_ue.tile([P, QW], dt.float32, tag="ue", name=f"ue_q{q}a{a}")

            def emit_epilogue(a, q=q, psc=psc, pues=pues):
                # tanh (ScalarE, overlaps next group's matmuls) then a
                # deferred M=1 score matmul
                t_sb = t_pool.tile([P, QW], mm_dt, tag="t", name=f"t_q{q}a{a}")
                nc.scalar.activation(
                    t_sb[:], pues[a][:, 0:QW],
                    mybir.ActivationFunctionType.Tanh,
                    bias=bias_sb[:, a : a + 1],
                )

                def score_mm(a=a, t_sb=t_sb, psc=psc):
                    nc.tensor.matmul(
                        psc[:], v_sb[:, a : a + 1], t_sb[:],
                        start=(a == 0), stop=(a == AT - 1),
                    )

                deferred.append(score_mm)

            if q == 0:
                # chunk 0: kt-outer over two a-subsets (only 5 PSUM banks =
                # 5 concurrently-pending accumulation groups), matching the
                # streaming arrival of u/encT
                for alist in ((0, 1, 2, 3, 4), (5, 6, 7)):
                    for kt in range(KT):
                        for a in alist:
                            mm_group(pues[a], et, a, kt)
                    for a in alist:
                        emit_epilogue(a)
            else:
                # dense chunks: a-outer, score matmuls one group behind
                for a in range(AT):
                    for kt in range(KT):
                        mm_group(pues[a], et, a, kt)
                    emit_epilogue(a)
                    if a >= 1:
                        flush_deferred()
            flush_deferred()

            # p = exp(scores - c)
            nc.scalar.activation(
                p_row[0:1, q * QW : (q + 1) * QW], psc[:],
                mybir.ActivationFunctionType.Exp, bias=negc_sb[0:1, 0:1],
            )
            pbc = pbc_pool.tile([P, QW], dt.float32, tag="pbc", name=f"pbc_q{q}")
            nc.gpsimd.partition_broadcast(pbc[:], p_row[0:1, q * QW : (q + 1) * QW])
            # partial context for this chunk: ctx[:, q*KT+kt] = sum_s et * p
            for kt in range(KT):
                nc.vector.affine_mul_reduce(
                    out=scratch[kt % 2][:],
                    accum_out=ctx_sb[:, q * KT + kt : q * KT + kt + 1],
                    in0=et[:, kt * QW : (kt + 1) * QW].bitcast(dt.float32),
                    in1=pbc[:],
                    scale=1.0,
                    bias=0.0,
                )
            nc.sync.dma_start(
                ctx_o[:, q * KT : (q + 1) * KT], ctx_sb[:, q * KT : (q + 1) * KT]
            )

        l_sb = out_pool.tile([1, 1], dt.float32)
        nc.vector.reduce_sum(l_sb[:], p_row[:], axis=mybir.AxisListType.X)
        nc.sync.dma_start(l_o[:], l_sb[:])

    nc.compile()
    return nc


def _get_nc(mode):
    if mode not in _COMPILED:
        _COMPILED[mode] = _build(mode)
    return _COMPILED[mode]


def kernel(**inputs):
    global LAST_EXEC_NS, LAST_RESULTS
    from concourse.bass_utils import run_bass_kernel_spmd

    enc = np.ascontiguousarray(np.asarray(inputs["encoder_outputs"], dtype=np.float32))
    hidden = np.asarray(inputs["hidden"], dtype=np.float32)
    U_w = np.asarray(inputs["U_w"], dtype=np.float32)
    U_b = np.asarray(inputs["U_b"], dtype=np.float32)
    W_w = np.asarray(inputs["W_w"], dtype=np.float32)
    W_b = np.asarray(inputs["W_b"], dtype=np.float32)
    V_w = np.asarray(inputs["V_w"], dtype=np.float32)
    V_b = np.asarray(inputs["V_b"], dtype=np.float32)

    bias_full = (U_b + W_b + W_w @ hidden).astype(np.float32)  # [A]
    U_wT = np.ascontiguousarray(U_w.T)  # [H, A]
    u_r = np.ascontiguousarray(
        U_wT.reshape(KT, P, AT, P).transpose(1, 0, 2, 3)
    )  # [128, KT, AT, 128]
    bias_t = np.ascontiguousarray(bias_full.reshape(AT, P))
    v_t = np.ascontiguousarray(V_w.reshape(AT, P))
    # softmax shift: scores are bounded by sum|V_w| + |V_b| (tanh in [-1,1]);
    # only shift when the bound could overflow exp in fp32.
    c = float(max(0.0, np.abs(V_w).sum() + abs(float(V_b[0])) - 30.0))
    negc = np.full((1, 1), -c, dtype=np.float32)

    in_maps = []
    for i in range(NCORES):
        shard = enc[i * S_LOC : (i + 1) * S_LOC]  # [S_LOC, H]
        # enc_q[q, p, kt*QW + s] = shard[q*QW + s, kt*128 + p]
        enc_t_i = np.ascontiguousarray(shard.T).reshape(KT, P, NQ, QW)
        enc_q_i = np.ascontiguousarray(
            enc_t_i.transpose(2, 1, 0, 3).reshape(NQ, P, KT * QW)
        )
        in_maps.append(
            {
                "enc_q": enc_q_i,
                "u_r": u_r,
                "bias_a": bias_t,
                "v_w": v_t,
                "neg_c": negc,
            }
        )

    nc = _get_nc(MODE)
    res = run_bass_kernel_spmd(nc, in_maps, list(range(NCORES)), trace=TRACE)
    LAST_EXEC_NS = res.exec_time_ns
    LAST_RESULTS = res

    ctx = np.zeros(H, dtype=np.float64)
    l = 0.0
    for i in range(NCORES):
        co = res.results[i]["ctx_out"].astype(np.float64)  # [128, NQ*KT]
        # ctx[kt*128 + p] = sum_q co[p, q*KT + kt]
        ctx += co.reshape(P, NQ, KT).sum(axis=1).T.reshape(H)
        l += float(res.results[i]["l_out"][0, 0])
    out = (ctx / l).astype(np.float32).reshape(1, 1, H)
    return out


# revision 16
# speedup vs baseline: 1.0655x; 1.0655x over previous
"""Bahdanau additive attention kernel for Trainium2, data-parallel over 8 NeuronCores.

reference:
    W_hidden = hidden @ W_w.T + W_b                      # [A]
    U_encode = encoder_outputs @ U_w.T + U_b             # [S, A]
    poly_tanh = tanh(U_encode + W_hidden)                # [S, A]
    scores = poly_tanh @ V_w.T + V_b                     # [S, 1]
    attn = softmax(scores, axis=0)
    context = attn.T @ encoder_outputs                   # -> [1, 1, H]

Sharding: encoder_outputs split along S across the 8 cores; the small
U/W/V weights are replicated.  Each core emits an unnormalized partial
context sum(p_s * enc[s, :]) and partial denominator sum(p_s) with
p_s = exp(score_s - c) for a host-chosen constant shift c (softmax is
shift-invariant, so V_b is dropped and c only guards against overflow;
tanh bounds |score| <= sum|V_w|, so c is computable on the host without
touching the scores).  The host reduces the 8 partials — the one
"all-reduce" this graph needs.

Per-core structure (S_loc = 1024, split into 4 s-chunks of 256):
  U_wT stays SBUF-resident ([128h, kt*a*128] = 16.8 MB).  Its 32 per-kt
  DMAs interleave with chunk 0's encT so the PE ramps while weights
  stream (chunk 0 walks kt-outer to match arrival order; later chunks
  walk a-outer and are fully dense).  encT is streamed per chunk in a
  host-packed [128p, kt*256s] layout (32 KB contiguous DMA lines).
  Per chunk: 256 f32r matmuls (K=h on partitions, N=256 => 1 cycle/row)
  accumulate UE[a] in per-a single-bank PSUM tiles; bias+tanh fused on
  ScalarE (a on partitions => per-partition bias); scores via M=1 f32r
  matmuls software-pipelined one a-group behind the main stream; exp on
  ScalarE; p broadcast across partitions on GpSimd; context = per-kt
  fused multiply-reduce (affine_mul_reduce) on VectorE against the
  chunk's encT, hidden under the next chunk's matmuls.  Only the last
  chunk's softmax+context is tail-exposed.
"""

import os
import sys
from contextlib import ExitStack

if "/opt/trn_rl_repo" not in sys.path:
    sys.path.insert(0, "/opt/trn_rl_repo")

import numpy as np

S, H, A, NCORES = 8192, 4096, 1024, 8
S_LOC = S // NCORES  # 1024
P = 128
KT = H // P  # 32 k-tiles (contraction over h)
AT = A // P  # 8 a-tiles
NQ = 4  # s-chunks per core
QW = S_LOC // NQ  # 256 (>= 256 keeps f32r matmuls at 1 cycle/row)

MODE = os.environ.get("NN_ATTN_MODE", "f32r")  # f32r | f32
TRACE = bool(int(os.environ.get("NN_ATTN_TRACE", "0")))

LAST_EXEC_NS = None
LAST_RESULTS = None

_COMPILED = {}


def _build(mode):
    import concourse.bacc as bacc
    import concourse.mybir as mybir
    import concourse.tile as tile

    dt = mybir.dt
    mm_dt = dt.float32r if mode == "f32r" else dt.float32

    nc = bacc.Bacc("TRN2", target_bir_lowering=False, debug=False)

    # enc_q[q][p][kt*QW + s] = enc[q*QW + s, kt*128 + p] — per-chunk slabs
    # with 32KB-contiguous per-partition DMA lines.
    enc_q = nc.dram_tensor("enc_q", [NQ, P, KT * QW], mm_dt, kind="ExternalInput").ap()
    u_r = nc.dram_tensor("u_r", [P, KT, AT, P], mm_dt, kind="ExternalInput").ap()
    bias_a = nc.dram_tensor("bias_a", [AT, P], dt.float32, kind="ExternalInput").ap()
    v_w = nc.dram_tensor("v_w", [AT, P], mm_dt, kind="ExternalInput").ap()
    neg_c = nc.dram_tensor("neg_c", [1, 1], dt.float32, kind="ExternalInput").ap()
    # ctx_out[p][q*KT + kt] = partial ctx of chunk q for h = kt*128 + p
    ctx_o = nc.dram_tensor("ctx_out", [P, NQ * KT], dt.float32, kind="ExternalOutput").ap()
    l_o = nc.dram_tensor("l_out", [1, 1], dt.float32, kind="ExternalOutput").ap()

    with tile.TileContext(nc) as tc, ExitStack() as ctx:
        const = ctx.enter_context(tc.tile_pool(name="const", bufs=1))
        u_pool = ctx.enter_context(tc.tile_pool(name="u", bufs=1))
        et_pool = ctx.enter_context(tc.tile_pool(name="et", bufs=2))
        t_pool = ctx.enter_context(tc.tile_pool(name="t", bufs=3))
        pbc_pool = ctx.enter_context(tc.tile_pool(name="pbc", bufs=2))
        out_pool = ctx.enter_context(tc.tile_pool(name="out", bufs=1))
        ps_ue = ctx.enter_context(tc.tile_pool(name="ps_ue", bufs=6, space="PSUM"))
        ps_sc = ctx.enter_context(tc.tile_pool(name="ps_sc", bufs=2, space="PSUM"))

        bias_sb = const.tile([P, AT], dt.float32)
        v_sb = const.tile([P, AT], mm_dt)
        negc_sb = const.tile([1, 1], dt.float32)

        p_row = const.tile([1, S_LOC], dt.float32)
        ctx_sb = out_pool.tile([P, NQ * KT], dt.float32)
        scratch = [
            out_pool.tile([P, QW], dt.float32, name=f"scratch{i}") for i in range(2)
        ]
        scratch2 = out_pool.tile([P, QW], dt.float32)

        # resident U_wT [128h, (kt, a, 128a)]: 32 per-kt DMAs interleaved
        # with chunk 0's encT sub-loads on the SP queues, so data needed by
        # the kt-outer chunk-0 sweep arrives in consumption order.
        u_sb = u_pool.tile([P, KT * AT * P], mm_dt)
        et0 = et_pool.tile([P, KT * QW], mm_dt, tag="et", name="et_q0")
        for kt in range(KT):
            nc.sync.dma_start(u_sb[:, kt * AT * P : (kt + 1) * AT * P], u_r[:, kt])
            nc.sync.dma_start(
                et0[:, kt * QW : (kt + 1) * QW],
                enc_q[0][:, kt * QW : (kt + 1) * QW],
            )
            if kt == 1:
                # small inputs ride along after the first two k-slabs
                nc.sync.dma_start(bias_sb[:], bias_a.rearrange("at p -> p at"))
                nc.sync.dma_start(v_sb[:], v_w.rearrange("at p -> p at"))
                nc.sync.dma_start(negc_sb[:], neg_c[:])

        def mm_group(pue, et, a, kt):
            nc.tensor.matmul(
                pue[:, 0:QW],
                u_sb[:, (kt * AT + a) * P : (kt * AT + a + 1) * P],
                et[:, kt * QW : (kt + 1) * QW],
                start=(kt == 0),
                stop=(kt == KT - 1),
            )

        deferred = []  # score-matmul thunks, flushed one a-group behind

        def flush_deferred():
            while deferred:
                deferred.pop(0)()

        for q in range(NQ):
            if q == 0:
                et = et0
            else:
                et = et_pool.tile([P, KT * QW], mm_dt, tag="et", name=f"et_q{q}")
                # later chunks prefetch on the Activation HWDGE queues, off
                # the critical SP stream
                nc.scalar.dma_start(et[:], enc_q[q])
            psc = ps_sc.tile([1, QW], dt.float32, tag="psc", name=f"psc_q{q}")
            pues = {}
            for a in range(AT):
                pues[a] = ps_ue.tile([P, QW], dt.float32, tag="ue", name=f"ue_q{q}a{a}")

            def emit_epilogue(a, q=q, psc=psc, pues=pues):
                # tanh (ScalarE, overlaps next group's matmuls) then a
                # deferred M=1 score matmul
                t_sb = t_pool.tile([P, QW], mm_dt, tag="t", name=f"t_q{q}a{a}")
                nc.scalar.activation(
                    t_sb[:], pues[a][:, 0:QW],
                    mybir.ActivationFunctionType.Tanh,
                    bias=bias_sb[:, a : a + 1],
                )

                def score_mm(a=a, t_sb=t_sb, psc=psc):
                    nc.tensor.matmul(
                        psc[:], v_sb[:, a : a + 1], t_sb[:],
                        start=(a == 0), stop=(a == AT - 1),
                    )

                deferred.append(score_mm)

            if q == 0:
                # chunk 0: kt-outer over two a-subsets (only 5 PSUM banks =
                # 5 concurrently-pending accumulation groups), matching the
                # streaming arrival of u/encT
                for alist in ((0, 1, 2, 3), (4, 5, 6, 7)):
                    for kt in range(KT):
                        for a in alist:
                            mm_group(pues[a], et, a, kt)
                    for a in alist:
                        emit_epilogue(a)
            else:
                # dense chunks: a-outer, score matmuls one group behind
                for a in range(AT):
                    for kt in range(KT):
                        mm_group(pues[a], et, a, kt)
                    emit_epilogue(a)
                    if a >= 1:
                        flush_deferred()
            flush_deferred()

            # p = exp(scores - c)
            nc.scalar.activation(
                p_row[0:1, q * QW : (q + 1) * QW], psc[:],
                mybir.ActivationFunctionType.Exp, bias=negc_sb[0:1, 0:1],
            )
            pbc = pbc_pool.tile([P, QW], dt.float32, tag="pbc", name=f"pbc_q{q}")
            nc.gpsimd.partition_broadcast(pbc[:], p_row[0:1, q * QW : (q + 1) * QW])
            # partial context for this chunk: ctx[:, q*KT+kt] = sum_s et * p
            for kt in range(KT):
                nc.vector.affine_mul_reduce(
                    out=scratch[kt % 2][:],
                    accum_out=ctx_sb[:, q * KT + kt : q * KT + kt + 1],
                    in0=et[:, kt * QW : (kt + 1) * QW].bitcast(dt.float32),
                    in1=pbc[:],
                    scale=1.0,
                    bias=0.0,
                )
            nc.sync.dma_start(
                ctx_o[:, q * KT : (q + 1) * KT], ctx_sb[:, q * KT : (q + 1) * KT]
            )

        l_sb = out_pool.tile([1, 1], dt.float32)
        nc.vector.reduce_sum(l_sb[:], p_row[:], axis=mybir.AxisListType.X)
        nc.sync.dma_start(l_o[:], l_sb[:])

    nc.compile()
    return nc


def _get_nc(mode):
    if mode not in _COMPILED:
        _COMPILED[mode] = _build(mode)
    return _COMPILED[mode]


def kernel(**inputs):
    global LAST_EXEC_NS, LAST_RESULTS
    from concourse.bass_utils import run_bass_kernel_spmd

    enc = np.ascontiguousarray(np.asarray(inputs["encoder_outputs"], dtype=np.float32))
    hidden = np.asarray(inputs["hidden"], dtype=np.float32)
    U_w = np.asarray(inputs["U_w"], dtype=np.float32)
    U_b = np.asarray(inputs["U_b"], dtype=np.float32)
    W_w = np.asarray(inputs["W_w"], dtype=np.float32)
    W_b = np.asarray(inputs["W_b"], dtype=np.float32)
    V_w = np.asarray(inputs["V_w"], dtype=np.float32)
    V_b = np.asarray(inputs["V_b"], dtype=np.float32)

    bias_full = (U_b + W_b + W_w @ hidden).astype(np.float32)  # [A]
    U_wT = np.ascontiguousarray(U_w.T)  # [H, A]
    u_r = np.ascontiguousarray(
        U_wT.reshape(KT, P, AT, P).transpose(1, 0, 2, 3)
    )  # [128, KT, AT, 128]
    bias_t = np.ascontiguousarray(bias_full.reshape(AT, P))
    v_t = np.ascontiguousarray(V_w.reshape(AT, P))
    # softmax shift: scores are bounded by sum|V_w| + |V_b| (tanh in [-1,1]);
    # only shift when the bound could overflow exp in fp32.
    c = float(max(0.0, np.abs(V_w).sum() + abs(float(V_b[0])) - 30.0))
    negc = np.full((1, 1), -c, dtype=np.float32)

    in_maps = []
    for i in range(NCORES):
        shard = enc[i * S_LOC : (i + 1) * S_LOC]  # [S_LOC, H]
        # enc_q[q, p, kt*QW + s] = shard[q*QW + s, kt*128 + p]
        enc_t_i = np.ascontiguousarray(shard.T).reshape(KT, P, NQ, QW)
        enc_q_i = np.ascontiguousarray(
            enc_t_i.transpose(2, 1, 0, 3).reshape(NQ, P, KT * QW)
        )
        in_maps.append(
            {
                "enc_q": enc_q_i,
                "u_r": u_r,
                "bias_a": bias_t,
                "v_w": v_t,
                "neg_c": negc,
            }
        )

    nc = _get_nc(MODE)
    res = run_bass_kernel_spmd(nc, in_maps, list(range(NCORES)), trace=TRACE)
    LAST_EXEC_NS = res.exec_time_ns
    LAST_RESULTS = res

    ctx = np.zeros(H, dtype=np.float64)
    l = 0.0
    for i in range(NCORES):
        co = res.results[i]["ctx_out"].astype(np.float64)  # [128, NQ*KT]
        # ctx[kt*128 + p] = sum_q co[p, q*KT + kt]
        ctx += co.reshape(P, NQ, KT).sum(axis=1).T.reshape(H)
        l += float(res.results[i]["l_out"][0, 0])
    out = (ctx / l).astype(np.float32).reshape(1, 1, H)
    return out
